# revision 1
# baseline (speedup 1.0000x reference)
"""Trainium2 Bass kernel for nn_CellListComputer (cell-list construction).

Self-contained: builds one SPMD Bass program, runs it on 8 NeuronCores via
run_bass_kernel_spmd, returns the full outputs:
  (frac, atom_flat_index, neighbor_flat, neighbor_trans, count, cumcount)

Strategy
--------
Data-parallel over atoms: each of the 8 cores gets 125,000 atoms, padded to
128*977 and laid out partition-major ([128, 977, 3] per core).

Per-atom outputs are computed with elementwise engine ops (no gathers):
  * frac = x/200 computed exactly (multiply by round(1/200) + Sterbenz-exact
    residual correction; 200 = 128+64+8 makes each subtraction exact).
  * bucket digits via round-to-nearest int cast + compare fixup (= floor).
  * the reference's padded-grid gather reduces to per-axis mod arithmetic,
    and the translation-case table is multilinear in the 6 boundary
    indicators (coefficients fitted at build time).

The segment_sum histogram is built on TensorE: per 128-atom column, fp16
one-hots of hi=flat//429 (as stationary operand) and lo=flat%429 (moving)
multiply-accumulate into one PSUM bank, giving the joint [128,429] count
table after 977 matmuls. Counts are AllReduced across the 8 cores, and the
exclusive cumsum is computed with a row-reduce + strict-lower-triangular
matmul + a tensor_tensor_scan along the free dim.
"""
import numpy as np
from contextlib import ExitStack

import concourse.bacc as bacc
import concourse.mybir as mybir
import concourse.tile as tile

AOT = mybir.AluOpType
AFT = mybir.ActivationFunctionType
F32, F16, I32 = mybir.dt.float32, mybir.dt.float16, mybir.dt.int32

G = 38
TOTAL = G ** 3            # 54872
HI = 128
LO = 429                  # HI*LO = 54912 >= TOTAL
RINV = float(np.float32(1.0) / np.float32(200.0))
RINV429 = float(np.float32(1.0) / np.float32(429.0))
N_ATOMS = 1_000_000
N_CORES = 8
COLS = 977                # ceil(125000/128)
CHUNK = 256

VEC_DISP = [(-1, 0, 0), (-1, -1, 0), (0, -1, 0), (1, -1, 0),
            (-1, 1, -1), (0, 1, -1), (1, 1, -1), (-1, 0, -1),
            (0, 0, -1), (1, 0, -1), (-1, -1, -1), (0, -1, -1),
            (1, -1, -1)]


def _trans_case(ux, uy, uz):
    if uz == 2:
        return 0
    if uz == 0:
        return 11 + ux - 3 * uy
    if uy == 0:
        return 2 + ux
    if uy == 2:
        return 14 + ux
    return {0: 1, 1: 0, 2: 17}[ux]


def _trans_terms():
    out = []
    for (dx, dy, dz) in VEC_DISP:
        axes = []
        if dx != 0:
            axes.append(('x', dx, 'lx' if dx < 0 else 'hx'))
        if dy != 0:
            axes.append(('y', dy, 'ly' if dy < 0 else 'hy'))
        if dz != 0:
            axes.append(('z', dz, 'lz' if dz < 0 else 'hz'))
        n = len(axes)
        fvals = {}
        for bits in range(1 << n):
            u = {'x': 1, 'y': 1, 'z': 1}
            for i, (ax_, d, _) in enumerate(axes):
                if (bits >> i) & 1:
                    u[ax_] = 0 if d < 0 else 2
            fvals[bits] = _trans_case(u['x'], u['y'], u['z'])
        terms = []
        for S in range(1, 1 << n):
            c, T = 0, S
            while True:
                c += (-1) ** bin(S ^ T).count('1') * fvals[T]
                if T == 0:
                    break
                T = (T - 1) & S
            if c != 0:
                feat = '*'.join(axes[i][2] for i in range(n) if (S >> i) & 1)
                terms.append((feat, c))
        out.append(terms)
    return out


def _nf_terms():
    out = []
    for (dx, dy, dz) in VEC_DISP:
        c = 1444 * dx + 38 * dy + dz
        terms = []
        if dx == -1:
            terms.append(('lx', 54872))
        elif dx == 1:
            terms.append(('hx', -54872))
        if dy == -1:
            terms.append(('ly', 1444))
        elif dy == 1:
            terms.append(('hy', -1444))
        if dz == -1:
            terms.append(('lz', 38))
        elif dz == 1:
            terms.append(('hz', -38))
        out.append((c, terms))
    return out


TRANS_TERMS = _trans_terms()
NF_TERMS = _nf_terms()


def build_program(cols=COLS, chunk=CHUNK, npc=N_ATOMS // N_CORES,
                  n_cores=N_CORES, repeat=1):
    nc_b = bacc.Bacc()
    x_ext = nc_b.declare_dram_parameter("x", [128, cols, 3], F32, isOutput=False)
    frac_ext = nc_b.declare_dram_parameter("frac", [128, cols, 3], F32, isOutput=True)
    flat_ext = nc_b.declare_dram_parameter("flat", [128, cols], I32, isOutput=True)
    nf_ext = nc_b.declare_dram_parameter("nf", [128, cols, 13], I32, isOutput=True)
    tr_ext = nc_b.declare_dram_parameter("tr", [128, cols, 13], I32, isOutput=True)
    cnt_ext = nc_b.declare_dram_parameter("cnt", [128, LO], I32, isOutput=True)
    cum_ext = nc_b.declare_dram_parameter("cum", [128, LO], I32, isOutput=True)

    hist_dram = nc_b.dram_tensor("hist_local", [128, LO], F32)
    hist_red = nc_b.dram_tensor("hist_red", [128, LO], F32, addr_space="Shared")

    n_dummy = 128 * cols - npc
    assert 0 <= n_dummy < cols, (n_dummy, cols, npc)

    with tile.TileContext(nc_b) as tc, ExitStack() as ctx:
        nc = tc.nc
        const = ctx.enter_context(tc.tile_pool(name="const", bufs=1))
        io_pool = ctx.enter_context(tc.tile_pool(name="io", bufs=2))
        work = ctx.enter_context(tc.tile_pool(name="work", bufs=2))
        oh_pool = ctx.enter_context(tc.tile_pool(name="oh", bufs=8))
        psum = ctx.enter_context(tc.tile_pool(name="ps", bufs=1, space="PSUM"))

        iota_lo = const.tile([128, LO], F16, tag="iota_lo")
        iota_hi = const.tile([128, HI], F16, tag="iota_hi")
        it32 = const.tile([128, LO], I32, tag="itmp")
        nc.gpsimd.iota(it32[:], pattern=[[1, LO]], base=0, channel_multiplier=0)
        nc.vector.tensor_copy(iota_lo[:], it32[:])
        it32b = const.tile([128, HI], I32, tag="itmp2")
        nc.gpsimd.iota(it32b[:], pattern=[[1, HI]], base=0, channel_multiplier=0)
        nc.vector.tensor_copy(iota_hi[:], it32b[:])
        hist_ps = psum.tile([128, LO], F32, tag="hist")
        dmask = None
        if n_dummy > 0:
            dm32 = const.tile([128, n_dummy], I32, tag="dm32")
            nc.gpsimd.iota(dm32[:], pattern=[[0, n_dummy]], base=0, channel_multiplier=1)
            dmf = const.tile([128, n_dummy], F32, tag="dmf")
            nc.vector.tensor_copy(dmf[:], dm32[:])
            dmask = const.tile([128, n_dummy], F32, tag="dmask")
            nc.vector.tensor_scalar(dmask[:], dmf[:], 127.0, 999.0,
                                    op0=AOT.is_equal, op1=AOT.mult)

        n_chunks = (cols + chunk - 1) // chunk
        mm_idx = 0
        total_mm = cols * repeat
        for rep in range(repeat):
          for ci in range(n_chunks):
            c0 = ci * chunk
            F = min(chunk, cols - c0)
            x = io_pool.tile([128, F, 3], F32, tag="x")
            nc.sync.dma_start(x[:], x_ext[:, c0:c0 + F, :])
            # exact q = x/200
            q0 = work.tile([128, F, 3], F32, tag="q0")
            nc.scalar.activation(q0[:], x[:], AFT.Copy, scale=RINV)
            s1 = work.tile([128, F, 3], F32, tag="s1")
            nc.vector.scalar_tensor_tensor(s1[:], q0[:], -128.0, x[:], op0=AOT.mult, op1=AOT.add)
            nc.vector.scalar_tensor_tensor(s1[:], q0[:], -64.0, s1[:], op0=AOT.mult, op1=AOT.add)
            nc.vector.scalar_tensor_tensor(s1[:], q0[:], -8.0, s1[:], op0=AOT.mult, op1=AOT.add)
            q = work.tile([128, F, 3], F32, tag="q")
            nc.vector.scalar_tensor_tensor(q[:], s1[:], RINV, q0[:], op0=AOT.mult, op1=AOT.add)
            nc.sync.dma_start(frac_ext[:, c0:c0 + F, :], q[:])
            # digits v = floor(q*38)
            y = work.tile([128, F, 3], F32, tag="y")
            nc.scalar.activation(y[:], q[:], AFT.Copy, scale=float(G))
            yi = work.tile([128, F, 3], I32, tag="yi")
            nc.vector.tensor_copy(yi[:], y[:])
            vf = work.tile([128, F, 3], F32, tag="vf")
            nc.vector.tensor_copy(vf[:], yi[:])
            gt = work.tile([128, F, 3], F32, tag="gtm")
            nc.vector.tensor_tensor(gt[:], vf[:], y[:], op=AOT.is_gt)
            nc.vector.tensor_tensor(vf[:], vf[:], gt[:], op=AOT.subtract)
            lm = work.tile([128, F, 3], F32, tag="lm")
            nc.vector.tensor_scalar(lm[:], vf[:], 0.0, None, op0=AOT.is_equal)
            hm = work.tile([128, F, 3], F32, tag="hm")
            nc.vector.tensor_scalar(hm[:], vf[:], float(G - 1), None, op0=AOT.is_equal)

            t1 = work.tile([128, F], F32, tag="t1")
            nc.vector.scalar_tensor_tensor(t1[:], vf[:, :, 0], float(G), vf[:, :, 1], op0=AOT.mult, op1=AOT.add)
            flat = work.tile([128, F], F32, tag="flat")
            nc.vector.scalar_tensor_tensor(flat[:], t1[:], float(G), vf[:, :, 2], op0=AOT.mult, op1=AOT.add)
            nc.gpsimd.dma_start(flat_ext[:, c0:c0 + F], flat[:])

            feats = {'lx': lm[:, :, 0], 'hx': hm[:, :, 0], 'ly': lm[:, :, 1],
                     'hy': hm[:, :, 1], 'lz': lm[:, :, 2], 'hz': hm[:, :, 2]}
            prod_needed = set()
            for terms in TRANS_TERMS:
                for f, _ in terms:
                    if '*' in f:
                        prod_needed.add(f)
            prod_tiles = {}
            for pf in sorted(prod_needed, key=lambda s: (s.count('*'), s)):
                parts = pf.split('*')
                pt = work.tile([128, F], F32, tag="pr_" + pf.replace('*', '_'))
                if len(parts) == 2:
                    nc.vector.tensor_tensor(pt[:], feats[parts[0]], feats[parts[1]], op=AOT.mult)
                else:
                    base = prod_tiles[parts[0] + '*' + parts[1]]
                    nc.vector.tensor_tensor(pt[:], base[:], feats[parts[2]], op=AOT.mult)
                prod_tiles[pf] = pt

            def feat_ap(name):
                return prod_tiles[name][:] if '*' in name else feats[name]

            tr = work.tile([128, F, 13], F32, tag="tr")
            for k, terms in enumerate(TRANS_TERMS):
                dst = tr[:, :, k]
                if len(terms) == 1:
                    f0, cc = terms[0]
                    nc.vector.tensor_scalar(dst, feat_ap(f0), float(cc), None, op0=AOT.mult)
                    continue
                acc = work.tile([128, F], F32, tag="tracc")
                f0, cc0 = terms[0]
                nc.vector.tensor_scalar(acc[:], feat_ap(f0), float(cc0), None, op0=AOT.mult)
                for (ff, cc) in terms[1:-1]:
                    nc.vector.scalar_tensor_tensor(acc[:], feat_ap(ff), float(cc), acc[:], op0=AOT.mult, op1=AOT.add)
                ff, cc = terms[-1]
                nc.vector.scalar_tensor_tensor(dst, feat_ap(ff), float(cc), acc[:], op0=AOT.mult, op1=AOT.add)
            nc.gpsimd.dma_start(tr_ext[:, c0:c0 + F, :], tr[:])

            nf = work.tile([128, F, 13], F32, tag="nf")
            for k, (cst, terms) in enumerate(NF_TERMS):
                dst = nf[:, :, k]
                if not terms:
                    nc.vector.tensor_scalar(dst, flat[:], float(cst), None, op0=AOT.add)
                    continue
                acc = work.tile([128, F], F32, tag="nfacc")
                nc.vector.tensor_scalar(acc[:], flat[:], float(cst), None, op0=AOT.add)
                for i, (ff, cc) in enumerate(terms):
                    tgt = dst if i == len(terms) - 1 else acc[:]
                    nc.vector.scalar_tensor_tensor(tgt, feat_ap(ff), float(cc), acc[:], op0=AOT.mult, op1=AOT.add)
            nc.gpsimd.dma_start(nf_ext[:, c0:c0 + F, :], nf[:])

            # histogram digits
            hi_f = work.tile([128, F], F32, tag="hi_f")
            nc.scalar.activation(hi_f[:], flat[:], AFT.Copy, scale=RINV429)
            hii = work.tile([128, F], I32, tag="hii")
            nc.vector.tensor_copy(hii[:], hi_f[:])
            hif = work.tile([128, F], F32, tag="hif")
            nc.vector.tensor_copy(hif[:], hii[:])
            chk = work.tile([128, F], F32, tag="chk")
            nc.vector.tensor_scalar(chk[:], hif[:], 429.0, None, op0=AOT.mult)
            nc.vector.tensor_tensor(chk[:], chk[:], flat[:], op=AOT.is_gt)
            nc.vector.tensor_tensor(hif[:], hif[:], chk[:], op=AOT.subtract)
            lo_f = work.tile([128, F], F32, tag="lo_f")
            nc.vector.scalar_tensor_tensor(lo_f[:], hif[:], -429.0, flat[:], op0=AOT.mult, op1=AOT.add)
            if n_dummy > 0:
                d_lo = max(cols - n_dummy, c0)
                d_hi = c0 + F
                if d_lo < d_hi:
                    a0 = d_lo - c0
                    m0 = d_lo - (cols - n_dummy)
                    w = d_hi - d_lo
                    nc.vector.tensor_tensor(hif[:, a0:a0 + w], hif[:, a0:a0 + w],
                                            dmask[:, m0:m0 + w], op=AOT.add)
                    nc.vector.tensor_tensor(lo_f[:, a0:a0 + w], lo_f[:, a0:a0 + w],
                                            dmask[:, m0:m0 + w], op=AOT.add)
            for t in range(F):
                oh_lo = oh_pool.tile([128, LO], F16, tag="oh_lo")
                oh_hi = oh_pool.tile([128, HI], F16, tag="oh_hi")
                nc.vector.tensor_scalar(oh_lo[:], iota_lo[:], lo_f[:, t:t + 1], None, op0=AOT.is_equal)
                nc.vector.tensor_scalar(oh_hi[:], iota_hi[:], hif[:, t:t + 1], None, op0=AOT.is_equal)
                nc.tensor.matmul(hist_ps[:], oh_hi[:], oh_lo[:],
                                 start=(mm_idx == 0), stop=(mm_idx == total_mm - 1))
                mm_idx += 1

        # reduce + cumsum
        hist_sb = const.tile([128, LO], F32, tag="hist_sb")
        nc.vector.tensor_copy(hist_sb[:], hist_ps[:])
        nc.sync.dma_start(hist_dram[:], hist_sb[:])
        nc.gpsimd.collective_compute(
            "AllReduce", AOT.add,
            replica_groups=[list(range(n_cores))],
            ins=[hist_dram[:]],
            outs=[hist_red[:]],
        )
        cnt = const.tile([128, LO], F32, tag="cnt")
        nc.sync.dma_start(cnt[:], hist_red[:])
        nc.gpsimd.dma_start(cnt_ext[:], cnt[:])
        rowsum = const.tile([128, 1], F32, tag="rowsum")
        nc.vector.tensor_reduce(rowsum[:], cnt[:], axis=mybir.AxisListType.X, op=AOT.add)
        lt32 = const.tile([128, 128], I32, tag="lt32")
        nc.gpsimd.iota(lt32[:], pattern=[[1, 128]], base=0, channel_multiplier=-1)
        ltf = const.tile([128, 128], F32, tag="ltf")
        nc.vector.tensor_copy(ltf[:], lt32[:])
        ltri = const.tile([128, 128], F32, tag="ltri")
        nc.vector.tensor_scalar(ltri[:], ltf[:], 0.0, None, op0=AOT.is_gt)
        rp_ps = psum.tile([128, 1], F32, tag="rp")
        nc.tensor.matmul(rp_ps[:], ltri[:], rowsum[:], start=True, stop=True)
        rowpre = const.tile([128, 1], F32, tag="rowpre")
        nc.vector.tensor_copy(rowpre[:], rp_ps[:])
        csh = const.tile([128, LO + 1], F32, tag="csh")
        nc.vector.memset(csh[:, 0:1], 0.0)
        nc.vector.tensor_copy(csh[:, 1:LO + 1], cnt[:])
        ones = const.tile([128, LO], F32, tag="ones")
        nc.vector.memset(ones[:], 1.0)
        within = const.tile([128, LO], F32, tag="within")
        nc.vector.tensor_tensor_scan(within[:], ones[:], csh[:, 0:LO], 0.0,
                                     op0=AOT.mult, op1=AOT.add)
        cum = const.tile([128, LO], F32, tag="cum")
        nc.vector.tensor_scalar(cum[:], within[:], rowpre[:], None, op0=AOT.add)
        nc.gpsimd.dma_start(cum_ext[:], cum[:])

    nc_b.finalize()
    return nc_b


_PROGRAM_CACHE = {}


def _get_program(repeat=1):
    key = repeat
    if key not in _PROGRAM_CACHE:
        _PROGRAM_CACHE[key] = build_program(repeat=repeat)
    return _PROGRAM_CACHE[key]


def make_in_maps(coordinates):
    coords = np.ascontiguousarray(np.asarray(coordinates), dtype=np.float32)
    npc = N_ATOMS // N_CORES
    pad = 128 * COLS
    in_maps = []
    for s in range(N_CORES):
        buf = np.zeros((pad, 3), np.float32)
        buf[:npc] = coords[0, s * npc:(s + 1) * npc]
        in_maps.append({"x": buf.reshape(128, COLS, 3)})
    return in_maps


def kernel(coordinates, cell):
    """Full inputs in, full outputs out (matches reference.reference)."""
    from concourse.bass_utils import run_bass_kernel_spmd

    coords = np.asarray(coordinates)
    assert coords.shape == (1, N_ATOMS, 3), coords.shape
    cell = np.asarray(cell)
    assert float(cell[0, 0]) == 200.0 and float(cell[1, 1]) == 200.0 \
        and float(cell[2, 2]) == 200.0, "kernel specialized for cell edge 200"

    in_maps = make_in_maps(coords)
    prog = _get_program()
    last_err = None
    for _attempt in range(2):
        try:
            res = run_bass_kernel_spmd(prog, in_maps, list(range(N_CORES)))
            break
        except Exception as e:  # transient device-unrecoverable: retry once
            last_err = e
    else:
        raise last_err

    npc = N_ATOMS // N_CORES
    pad = 128 * COLS
    frac = np.empty((1, N_ATOMS, 3), np.float32)
    flat = np.empty((1, N_ATOMS), np.int32)
    nf = np.empty((1, N_ATOMS, 13), np.int32)
    tr = np.empty((1, N_ATOMS, 13), np.int32)
    for s in range(N_CORES):
        r = res.results[s]
        sl = slice(s * npc, (s + 1) * npc)
        frac[0, sl] = r["frac"].reshape(pad, 3)[:npc]
        flat[0, sl] = r["flat"].reshape(pad)[:npc]
        nf[0, sl] = r["nf"].reshape(pad, 13)[:npc]
        tr[0, sl] = r["tr"].reshape(pad, 13)[:npc]
    cnt = res.results[0]["cnt"].reshape(-1)[:TOTAL].astype(np.int32)
    cum = res.results[0]["cum"].reshape(-1)[:TOTAL].astype(np.int32)
    return frac, flat, nf, tr, cnt, cum
